# revision 23
# baseline (speedup 1.0000x reference)
"""ExpandingLinear (sparse EmbedLinear + sparse ExpandingLinear tail) on 8 trn2 cores.

Math:
    h  = relu(x @ W_e.T)          W_e sparse [R_EMB, F_IN]  (COO, 6.25% dense)
    x2 = concat([x, h], axis=1)
    y  = x2 @ W.T + bias          W   sparse [F_OUT, F_MID], bias sparse [F_OUT]

Strategy: densify the sparse weights on the host (one-time weight prep,
O(nnz) work), then run the O(nnz * B) compute as two dense matmuls on the
TensorEngine.  Data-parallel over the batch: each of the 8 cores gets
B/8 = 256 rows of x (as x.T columns) and the full dense weights.

Device layout (per core), all transposed so batch is the matmul free dim:
    xt_sb  [128, 8*256]      x.T tiles      (f-major)
    we/wt  [128, G*1024] x k W_e.T / W.T row-block stripes (~1 MB DMAs)
    MM1: psum_h[r] += we[f,r].T @ xt[f]   -> relu -> hT[r]  [128, 256]
    MM2: psum_o[o] += wt[c,o].T @ x2t[c]  (x2t = xt tiles ++ hT tiles)
    out: outT[o] = psum_o[o] + bias[o]    -> DRAM [1024, 256]

Modes (KERNEL_MODE env): "bf16" (default, halves DMA bytes) or "f32r"
(fp32 storage, full-rate fp32r matmuls, ~40x lower error).
"""

import os

import numpy as np

B = 2048
F_IN = 1024
R_EMB = 1024
F_OUT = 1024
F_MID = F_IN + R_EMB
N_CORES = 8
B_SH = B // N_CORES  # 256

P = 128
NF = F_IN // P    # 8 f-tiles (MM1 contraction)
NR = R_EMB // P   # 8 r-tiles (MM1 outputs / psum tiles)
NC_T = F_MID // P  # 16 c-tiles (MM2 contraction)
NO = F_OUT // P   # 8 o-tiles (MM2 outputs)

_cache = {}


def _split_excess_waits(nc, mybir, max_waits=1):
    """Walrus in this container rejects instructions with >1 sync waits
    ("Too many sync wait commands").  Hoist excess waits onto same-engine
    NOPs placed immediately before the offending instruction."""
    cnt = 0
    for f in nc.m.functions:
        for b in f.blocks:
            out = []
            for inst in b.instructions:
                si = inst.sync_info
                if si is not None and len(si.on_wait) > max_waits:
                    waits = list(si.on_wait)
                    keep = waits[-max_waits:]
                    hoist = waits[:-max_waits]
                    for j in range(0, len(hoist), max_waits):
                        chunk = hoist[j : j + max_waits]
                        out.append(
                            mybir.InstNoOp(
                                name=f"{inst.name}_splitw{j}",
                                engine=inst.engine,
                                sync_info=mybir.SyncInfo(on_wait=chunk, on_update=[]),
                                bass_nofuse=True,
                            )
                        )
                        cnt += 1
                    inst.sync_info = mybir.SyncInfo(
                        on_wait=keep, on_update=list(si.on_update)
                    )
                out.append(inst)
            b.instructions = out
    return cnt


def _build(mode):
    import concourse.bass as bass
    import concourse.mybir as mybir
    import concourse.tile as tile

    dt = mybir.dt
    mm_dt = dt.float32r if mode == "f32r" else dt.bfloat16
    G = 2  # row-blocks per weight stripe

    nc = bass.Bass("TRN2", target_bir_lowering=False, debug=False, num_devices=N_CORES)

    xT = nc.declare_dram_parameter("xT", [F_IN, B_SH], mm_dt, isOutput=False)
    weT = nc.declare_dram_parameter("weT", [F_IN, R_EMB], mm_dt, isOutput=False)
    wT = nc.declare_dram_parameter("wT", [F_MID, F_OUT], mm_dt, isOutput=False)
    biasT = nc.declare_dram_parameter("biasT", [P, NO], dt.float32, isOutput=False)
    outT = nc.declare_dram_parameter("outT", [F_OUT, B_SH], dt.float32, isOutput=True)

    with tile.TileContext(nc) as tc:
        with (
            tc.tile_pool(name="xt", bufs=1) as xt_pool,
            tc.tile_pool(name="w", bufs=(NF + NC_T) // G) as w_pool,
            tc.tile_pool(name="h", bufs=NR) as h_pool,
            tc.tile_pool(name="ot", bufs=NO) as out_pool,
            tc.tile_pool(name="bias", bufs=1) as bias_pool,
            tc.tile_pool(name="psum", bufs=8, space="PSUM") as psum_pool,
        ):
            # x.T resident: [128, f-major 8*256]
            xt_sb = xt_pool.tile([P, NF * B_SH], mm_dt)
            nc.sync.dma_start(
                out=xt_sb[:].rearrange("p (f b) -> p f b", b=B_SH),
                in_=xT[:].rearrange("(f p) b -> p f b", p=P),
            )
            bias_sb = bias_pool.tile([P, NO], dt.float32)
            nc.scalar.dma_start(out=bias_sb[:], in_=biasT[:])

            # PE warm-up source: memset tile, ready long before any DMA lands
            wsrc = bias_pool.tile([P, B_SH], mm_dt, name="wsrc")
            nc.gpsimd.memset(wsrc[:], 0)

            # weight stripes on the sync ring only (a single HWDGE ring
            # sustains ~320 GB/s; splitting across two measured slower)
            we_t = []
            for k in range(NF // G):
                t = w_pool.tile([P, G * R_EMB], mm_dt, tag="w", name=f"we{k}")
                nc.sync.dma_start(
                    out=t[:].rearrange("p (f r) -> p f r", r=R_EMB),
                    in_=weT[G * k * P : G * (k + 1) * P, :].rearrange(
                        "(f p) r -> p f r", p=P
                    ),
                )
                we_t.append(t)
            wt_t = []
            for k in range(NC_T // G):
                t = w_pool.tile([P, G * F_OUT], mm_dt, tag="w", name=f"wt{k}")
                nc.sync.dma_start(
                    out=t[:].rearrange("p (c o) -> p c o", o=F_OUT),
                    in_=wT[G * k * P : G * (k + 1) * P, :].rearrange(
                        "(c p) o -> p c o", p=P
                    ),
                )
                wt_t.append(t)

            # MM1: psum_h[r] = sum_f we[f, r-block].T @ xt[f]
            psum_h = [
                psum_pool.tile([P, B_SH], dt.float32, tag="acc", name=f"ph{r}")
                for r in range(NR)
            ]

            # PE warm-up: ~4us of garbage matmuls starting right after the
            # memset so the HAM clock gate is at 2.4 GHz when real matmuls
            # start.  Results land in psum_h[0] partitions 0-7 and are wiped
            # by MM1's start=True.
            for _ in range(22):
                nc.tensor.matmul(
                    out=psum_h[0][0:8, :],
                    lhsT=wsrc[:, 0:8],
                    rhs=wsrc[:],
                    start=True,
                    stop=True,
                )
            for f in range(NF):
                lhs_base = we_t[f // G]
                off = (f % G) * R_EMB
                rhs = xt_sb[:, f * B_SH : (f + 1) * B_SH]
                for r in range(NR):
                    nc.tensor.matmul(
                        out=psum_h[r][:],
                        lhsT=lhs_base[:, off + r * P : off + (r + 1) * P],
                        rhs=rhs,
                        start=(f == 0),
                        stop=(f == NF - 1),
                    )

            hT = []
            for r in range(NR):
                t = h_pool.tile([P, B_SH], mm_dt, tag="h", name=f"h{r}")
                nc.vector.tensor_scalar_max(t[:], psum_h[r][:], 0.0)
                hT.append(t)

            # MM2: psum_o[o] = sum_c wt[c, o-block].T @ x2t[c]
            psum_o = [
                psum_pool.tile([P, B_SH], dt.float32, tag="acc", name=f"po{o}")
                for o in range(NO)
            ]
            for c in range(NC_T):
                lhs_base = wt_t[c // G]
                off = (c % G) * F_OUT
                if c < NF:
                    rhs = xt_sb[:, c * B_SH : (c + 1) * B_SH]
                else:
                    rhs = hT[c - NF][:]
                for o in range(NO):
                    nc.tensor.matmul(
                        out=psum_o[o][:],
                        lhsT=lhs_base[:, off + o * P : off + (o + 1) * P],
                        rhs=rhs,
                        start=(c == 0),
                        stop=(c == NC_T - 1),
                    )

            for o in range(NO):
                t = out_pool.tile([P, B_SH], dt.float32, tag="ot", name=f"ot{o}")
                if o % 2 == 0:
                    nc.vector.tensor_scalar_add(
                        t[:], psum_o[o][:], bias_sb[:, o : o + 1]
                    )
                else:
                    nc.scalar.activation(
                        t[:],
                        psum_o[o][:],
                        mybir.ActivationFunctionType.Identity,
                        bias=bias_sb[:, o : o + 1],
                    )
                # stores on the scalar ring: never behind residual load packets
                nc.scalar.dma_start(out=outT[o * P : (o + 1) * P, :], in_=t[:])

    _split_excess_waits(nc, mybir)
    return nc


def kernel(
    x,
    embed_rows,
    embed_cols,
    embed_vals,
    w_rows,
    w_cols,
    w_vals,
    bias_idx,
    bias_vals,
):
    from concourse.bass_utils import run_bass_kernel_spmd

    mode = os.environ.get("KERNEL_MODE", "bf16")

    # --- host-side weight prep (one-time, O(nnz)) --------------------------
    weT = (
        np.bincount(
            embed_cols.astype(np.int64) * R_EMB + embed_rows.astype(np.int64),
            weights=embed_vals.astype(np.float64),
            minlength=F_IN * R_EMB,
        )
        .reshape(F_IN, R_EMB)
        .astype(np.float32)
    )
    wT = (
        np.bincount(
            w_cols.astype(np.int64) * F_OUT + w_rows.astype(np.int64),
            weights=w_vals.astype(np.float64),
            minlength=F_MID * F_OUT,
        )
        .reshape(F_MID, F_OUT)
        .astype(np.float32)
    )
    bias = np.bincount(
        bias_idx.astype(np.int64), weights=bias_vals.astype(np.float64), minlength=F_OUT
    ).astype(np.float32)
    biasT = np.ascontiguousarray(bias.reshape(NO, P).T)

    xT = np.ascontiguousarray(x.T.astype(np.float32))
    if mode == "bf16":
        import ml_dtypes

        np_dt = ml_dtypes.bfloat16
        xT = xT.astype(np_dt)
        weT = weT.astype(np_dt)
        wT = wT.astype(np_dt)

    key = ("nc", mode)
    if key not in _cache:
        _cache[key] = _build(mode)
    nc = _cache[key]

    in_maps = [
        {
            "xT": np.ascontiguousarray(xT[:, j * B_SH : (j + 1) * B_SH]),
            "weT": weT,
            "wT": wT,
            "biasT": biasT,
        }
        for j in range(N_CORES)
    ]

    trace = bool(os.environ.get("KERNEL_TRACE"))
    kw = {}
    if trace:
        import concourse.bass_utils as bu

        bu.upload_artifacts = lambda t: t  # no artifact store in this container
        kw = dict(trace=True, tmpdir=os.environ.get("KERNEL_TRACE_DIR") or None)

    res = run_bass_kernel_spmd(nc, in_maps, list(range(N_CORES)), **kw)
    if trace:
        _cache["last_result"] = res

    out = np.empty((B, F_OUT), np.float32)
    for j in range(N_CORES):
        out[j * B_SH : (j + 1) * B_SH, :] = res.results[j]["outT"].T
    return out


# revision 30
# speedup vs baseline: 1.1118x; 1.1118x over previous
"""ExpandingLinear (sparse EmbedLinear + sparse ExpandingLinear tail) on 8 trn2 cores.

Math:
    h  = relu(x @ W_e.T)          W_e sparse [R_EMB, F_IN]  (COO, 6.25% dense)
    x2 = concat([x, h], axis=1)
    y  = x2 @ W.T + bias          W   sparse [F_OUT, F_MID], bias sparse [F_OUT]

Strategy: densify the sparse weights on the host (one-time weight prep,
O(nnz) work), then run the O(nnz * B) compute as two dense matmuls on the
TensorEngine.  Data-parallel over the batch: each of the 8 cores gets
B/8 = 256 rows of x (as x.T columns) and the full dense weights.

Device layout (per core), all transposed so batch is the matmul free dim:
    xt_sb  [128, 8*256]      x.T tiles      (f-major)
    we/wt  [128, G*1024] x k W_e.T / W.T row-block stripes (~1 MB DMAs)
    MM1: psum_h[r] += we[f,r].T @ xt[f]   -> relu -> hT[r]  [128, 256]
    MM2: psum_o[o] += wt[c,o].T @ x2t[c]  (x2t = xt tiles ++ hT tiles)
    out: outT[o] = psum_o[o] + bias[o]    -> DRAM [1024, 256]

Modes (KERNEL_MODE env): "bf16" (default, halves DMA bytes) or "f32r"
(fp32 storage, full-rate fp32r matmuls, ~40x lower error).
"""

import os

import numpy as np

B = 2048
F_IN = 1024
R_EMB = 1024
F_OUT = 1024
F_MID = F_IN + R_EMB
N_CORES = 8
B_SH = B // N_CORES  # 256

P = 128
NF = F_IN // P    # 8 f-tiles (MM1 contraction)
NR = R_EMB // P   # 8 r-tiles (MM1 outputs / psum tiles)
NC_T = F_MID // P  # 16 c-tiles (MM2 contraction)
NO = F_OUT // P   # 8 o-tiles (MM2 outputs)

_cache = {}


def _split_excess_waits(nc, mybir, max_waits=1):
    """Walrus in this container rejects instructions with >1 sync waits
    ("Too many sync wait commands").  Hoist excess waits onto same-engine
    NOPs placed immediately before the offending instruction."""
    cnt = 0
    for f in nc.m.functions:
        for b in f.blocks:
            out = []
            for inst in b.instructions:
                si = inst.sync_info
                if si is not None and len(si.on_wait) > max_waits:
                    waits = list(si.on_wait)
                    keep = waits[-max_waits:]
                    hoist = waits[:-max_waits]
                    for j in range(0, len(hoist), max_waits):
                        chunk = hoist[j : j + max_waits]
                        out.append(
                            mybir.InstNoOp(
                                name=f"{inst.name}_splitw{j}",
                                engine=inst.engine,
                                sync_info=mybir.SyncInfo(on_wait=chunk, on_update=[]),
                                bass_nofuse=True,
                            )
                        )
                        cnt += 1
                    inst.sync_info = mybir.SyncInfo(
                        on_wait=keep, on_update=list(si.on_update)
                    )
                out.append(inst)
            b.instructions = out
    return cnt


def _build(mode):
    import concourse.bass as bass
    import concourse.mybir as mybir
    import concourse.tile as tile

    dt = mybir.dt
    mm_dt = dt.float32r if mode == "f32r" else dt.bfloat16
    G1 = 1  # row-blocks per MM1 weight stripe (small: earliest PE start)
    G2 = 2  # row-blocks per MM2 weight stripe

    nc = bass.Bass("TRN2", target_bir_lowering=False, debug=False, num_devices=N_CORES)

    xT = nc.declare_dram_parameter("xT", [F_IN, B_SH], mm_dt, isOutput=False)
    weT = nc.declare_dram_parameter("weT", [F_IN, R_EMB], mm_dt, isOutput=False)
    wT = nc.declare_dram_parameter("wT", [F_MID, F_OUT], mm_dt, isOutput=False)
    biasT = nc.declare_dram_parameter("biasT", [P, NO], dt.float32, isOutput=False)
    outT = nc.declare_dram_parameter("outT", [F_OUT, B_SH], dt.float32, isOutput=True)

    with tile.TileContext(nc) as tc:
        with (
            tc.tile_pool(name="xt", bufs=1) as xt_pool,
            tc.tile_pool(name="w", bufs=8) as w_pool,
            tc.tile_pool(name="h", bufs=NR) as h_pool,
            tc.tile_pool(name="ot", bufs=NO) as out_pool,
            tc.tile_pool(name="bias", bufs=1) as bias_pool,
            tc.tile_pool(name="psum", bufs=8, space="PSUM") as psum_pool,
        ):
            # PE warm-up source: memset tile, ready long before any DMA lands
            wsrc = bias_pool.tile([P, B_SH], mm_dt, name="wsrc")
            nc.gpsimd.memset(wsrc[:], 0)

            # x.T + bias via SWDGE (gpsimd) — separate queues, starts before
            # the sync ring's preamble finishes
            xt_sb = xt_pool.tile([P, NF * B_SH], mm_dt)
            nc.gpsimd.dma_start(
                out=xt_sb[:].rearrange("p (f b) -> p f b", b=B_SH),
                in_=xT[:].rearrange("(f p) b -> p f b", p=P),
            )
            bias_sb = bias_pool.tile([P, NO], dt.float32)
            nc.gpsimd.dma_start(out=bias_sb[:], in_=biasT[:])

            # weight stripes on the sync ring only (a single HWDGE ring
            # sustains ~320 GB/s; splitting across two measured slower)
            we_t = []
            for k in range(NF // G1):
                t = w_pool.tile([P, G1 * R_EMB], mm_dt, tag="we", name=f"we{k}")
                nc.sync.dma_start(
                    out=t[:].rearrange("p (f r) -> p f r", r=R_EMB),
                    in_=weT[G1 * k * P : G1 * (k + 1) * P, :].rearrange(
                        "(f p) r -> p f r", p=P
                    ),
                )
                we_t.append(t)
            wt_t = []
            for k in range(NC_T // G2):
                t = w_pool.tile([P, G2 * F_OUT], mm_dt, tag="wt", name=f"wt{k}")
                nc.sync.dma_start(
                    out=t[:].rearrange("p (c o) -> p c o", o=F_OUT),
                    in_=wT[G2 * k * P : G2 * (k + 1) * P, :].rearrange(
                        "(c p) o -> p c o", p=P
                    ),
                )
                wt_t.append(t)

            # MM1: psum_h[r] = sum_f we[f, r-block].T @ xt[f]
            psum_h = [
                psum_pool.tile([P, B_SH], dt.float32, tag="acc", name=f"ph{r}")
                for r in range(NR)
            ]

            # PE warm-up: ~4us of garbage matmuls starting right after the
            # memset so the HAM clock gate is at 2.4 GHz when real matmuls
            # start.  Results land in psum_h[0] partitions 0-7 and are wiped
            # by MM1's start=True.
            for _ in range(26):
                nc.tensor.matmul(
                    out=psum_h[0][0:8, :],
                    lhsT=wsrc[:, 0:8],
                    rhs=wsrc[:],
                    start=True,
                    stop=True,
                )
            for f in range(NF):
                lhs_base = we_t[f // G1]
                off = (f % G1) * R_EMB
                rhs = xt_sb[:, f * B_SH : (f + 1) * B_SH]
                for r in range(NR):
                    nc.tensor.matmul(
                        out=psum_h[r][:],
                        lhsT=lhs_base[:, off + r * P : off + (r + 1) * P],
                        rhs=rhs,
                        start=(f == 0),
                        stop=(f == NF - 1),
                    )

            hT = []
            for r in range(NR):
                t = h_pool.tile([P, B_SH], mm_dt, tag="h", name=f"h{r}")
                nc.vector.tensor_scalar_max(t[:], psum_h[r][:], 0.0)
                hT.append(t)

            # MM2: psum_o[o] = sum_c wt[c, o-block].T @ x2t[c]
            psum_o = [
                psum_pool.tile([P, B_SH], dt.float32, tag="acc", name=f"po{o}")
                for o in range(NO)
            ]
            for c in range(NC_T):
                lhs_base = wt_t[c // G2]
                off = (c % G2) * F_OUT
                if c < NF:
                    rhs = xt_sb[:, c * B_SH : (c + 1) * B_SH]
                else:
                    rhs = hT[c - NF][:]
                for o in range(NO):
                    nc.tensor.matmul(
                        out=psum_o[o][:],
                        lhsT=lhs_base[:, off + o * P : off + (o + 1) * P],
                        rhs=rhs,
                        start=(c == 0),
                        stop=(c == NC_T - 1),
                    )

            for o in range(NO):
                t = out_pool.tile([P, B_SH], dt.float32, tag="ot", name=f"ot{o}")
                nc.vector.tensor_scalar_add(t[:], psum_o[o][:], bias_sb[:, o : o + 1])
                # alternate stores across the two HWDGE rings
                ring = nc.scalar if o % 2 == 0 else nc.sync
                ring.dma_start(out=outT[o * P : (o + 1) * P, :], in_=t[:])

    _split_excess_waits(nc, mybir)
    return nc


def kernel(
    x,
    embed_rows,
    embed_cols,
    embed_vals,
    w_rows,
    w_cols,
    w_vals,
    bias_idx,
    bias_vals,
):
    from concourse.bass_utils import run_bass_kernel_spmd

    mode = os.environ.get("KERNEL_MODE", "bf16")

    # --- host-side weight prep (one-time, O(nnz)) --------------------------
    weT = (
        np.bincount(
            embed_cols.astype(np.int64) * R_EMB + embed_rows.astype(np.int64),
            weights=embed_vals.astype(np.float64),
            minlength=F_IN * R_EMB,
        )
        .reshape(F_IN, R_EMB)
        .astype(np.float32)
    )
    wT = (
        np.bincount(
            w_cols.astype(np.int64) * F_OUT + w_rows.astype(np.int64),
            weights=w_vals.astype(np.float64),
            minlength=F_MID * F_OUT,
        )
        .reshape(F_MID, F_OUT)
        .astype(np.float32)
    )
    bias = np.bincount(
        bias_idx.astype(np.int64), weights=bias_vals.astype(np.float64), minlength=F_OUT
    ).astype(np.float32)
    biasT = np.ascontiguousarray(bias.reshape(NO, P).T)

    xT = np.ascontiguousarray(x.T.astype(np.float32))
    if mode == "bf16":
        import ml_dtypes

        np_dt = ml_dtypes.bfloat16
        xT = xT.astype(np_dt)
        weT = weT.astype(np_dt)
        wT = wT.astype(np_dt)

    key = ("nc", mode)
    if key not in _cache:
        _cache[key] = _build(mode)
    nc = _cache[key]

    in_maps = [
        {
            "xT": np.ascontiguousarray(xT[:, j * B_SH : (j + 1) * B_SH]),
            "weT": weT,
            "wT": wT,
            "biasT": biasT,
        }
        for j in range(N_CORES)
    ]

    trace = bool(os.environ.get("KERNEL_TRACE"))
    kw = {}
    if trace:
        import concourse.bass_utils as bu

        bu.upload_artifacts = lambda t: t  # no artifact store in this container
        kw = dict(trace=True, tmpdir=os.environ.get("KERNEL_TRACE_DIR") or None)

    res = run_bass_kernel_spmd(nc, in_maps, list(range(N_CORES)), **kw)
    if trace:
        _cache["last_result"] = res

    out = np.empty((B, F_OUT), np.float32)
    for j in range(N_CORES):
        out[j * B_SH : (j + 1) * B_SH, :] = res.results[j]["outT"].T
    return out


# revision 31
# speedup vs baseline: 1.2218x; 1.0989x over previous
"""ExpandingLinear (sparse EmbedLinear + sparse ExpandingLinear tail) on 8 trn2 cores.

Math:
    h  = relu(x @ W_e.T)          W_e sparse [R_EMB, F_IN]  (COO, 6.25% dense)
    x2 = concat([x, h], axis=1)
    y  = x2 @ W.T + bias          W   sparse [F_OUT, F_MID], bias sparse [F_OUT]

Strategy: densify the sparse weights on the host (one-time weight prep,
O(nnz) work), then run the O(nnz * B) compute as two dense matmuls on the
TensorEngine.  Data-parallel over the batch: each of the 8 cores gets
B/8 = 256 rows of x (as x.T columns) and the full dense weights.

Device layout (per core), all transposed so batch is the matmul free dim:
    xt_sb  [128, 8*256]      x.T tiles      (f-major)
    we/wt  [128, G*1024] x k W_e.T / W.T row-block stripes (~1 MB DMAs)
    MM1: psum_h[r] += we[f,r].T @ xt[f]   -> relu -> hT[r]  [128, 256]
    MM2: psum_o[o] += wt[c,o].T @ x2t[c]  (x2t = xt tiles ++ hT tiles)
    out: outT[o] = psum_o[o] + bias[o]    -> DRAM [1024, 256]

Modes (KERNEL_MODE env): "bf16" (default, halves DMA bytes) or "f32r"
(fp32 storage, full-rate fp32r matmuls, ~40x lower error).
"""

import os

import numpy as np

B = 2048
F_IN = 1024
R_EMB = 1024
F_OUT = 1024
F_MID = F_IN + R_EMB
N_CORES = 8
B_SH = B // N_CORES  # 256

P = 128
NF = F_IN // P    # 8 f-tiles (MM1 contraction)
NR = R_EMB // P   # 8 r-tiles (MM1 outputs / psum tiles)
NC_T = F_MID // P  # 16 c-tiles (MM2 contraction)
NO = F_OUT // P   # 8 o-tiles (MM2 outputs)

_cache = {}


def _split_excess_waits(nc, mybir, max_waits=1):
    """Walrus in this container rejects instructions with >1 sync waits
    ("Too many sync wait commands").  Hoist excess waits onto same-engine
    NOPs placed immediately before the offending instruction."""
    cnt = 0
    for f in nc.m.functions:
        for b in f.blocks:
            out = []
            for inst in b.instructions:
                si = inst.sync_info
                if si is not None and len(si.on_wait) > max_waits:
                    waits = list(si.on_wait)
                    keep = waits[-max_waits:]
                    hoist = waits[:-max_waits]
                    for j in range(0, len(hoist), max_waits):
                        chunk = hoist[j : j + max_waits]
                        out.append(
                            mybir.InstNoOp(
                                name=f"{inst.name}_splitw{j}",
                                engine=inst.engine,
                                sync_info=mybir.SyncInfo(on_wait=chunk, on_update=[]),
                                bass_nofuse=True,
                            )
                        )
                        cnt += 1
                    inst.sync_info = mybir.SyncInfo(
                        on_wait=keep, on_update=list(si.on_update)
                    )
                out.append(inst)
            b.instructions = out
    return cnt


def _build(mode):
    import concourse.bass as bass
    import concourse.mybir as mybir
    import concourse.tile as tile

    dt = mybir.dt
    mm_dt = dt.float32r if mode == "f32r" else dt.bfloat16
    G1 = 1  # row-blocks per MM1 weight stripe (small: earliest PE start)
    G2 = 2  # row-blocks per MM2 weight stripe

    nc = bass.Bass("TRN2", target_bir_lowering=False, debug=False, num_devices=N_CORES)

    xT = nc.declare_dram_parameter("xT", [F_IN, B_SH], mm_dt, isOutput=False)
    weT = nc.declare_dram_parameter("weT", [F_IN, R_EMB], mm_dt, isOutput=False)
    wT = nc.declare_dram_parameter("wT", [F_MID, F_OUT], mm_dt, isOutput=False)
    biasT = nc.declare_dram_parameter("biasT", [P, NO], dt.float32, isOutput=False)
    outT = nc.declare_dram_parameter("outT", [F_OUT, B_SH], dt.float32, isOutput=True)

    with tile.TileContext(nc) as tc:
        with (
            tc.tile_pool(name="xt", bufs=1) as xt_pool,
            tc.tile_pool(name="w", bufs=8) as w_pool,
            tc.tile_pool(name="h", bufs=NR) as h_pool,
            tc.tile_pool(name="ot", bufs=NO) as out_pool,
            tc.tile_pool(name="bias", bufs=1) as bias_pool,
            tc.tile_pool(name="psum", bufs=8, space="PSUM") as psum_pool,
        ):
            # PE warm-up source: memset tile, ready long before any DMA lands
            wsrc = bias_pool.tile([P, B_SH], mm_dt, name="wsrc")
            nc.gpsimd.memset(wsrc[:], 0)

            # x.T first on the sync ring (gates all of MM1); bias on scalar
            xt_sb = xt_pool.tile([P, NF * B_SH], mm_dt)
            nc.sync.dma_start(
                out=xt_sb[:].rearrange("p (f b) -> p f b", b=B_SH),
                in_=xT[:].rearrange("(f p) b -> p f b", p=P),
            )
            bias_sb = bias_pool.tile([P, NO], dt.float32)
            nc.scalar.dma_start(out=bias_sb[:], in_=biasT[:])

            # weight stripes on the sync ring only (a single HWDGE ring
            # sustains ~320 GB/s; splitting across two measured slower)
            we_t = []
            for k in range(NF // G1):
                t = w_pool.tile([P, G1 * R_EMB], mm_dt, tag="we", name=f"we{k}")
                nc.sync.dma_start(
                    out=t[:].rearrange("p (f r) -> p f r", r=R_EMB),
                    in_=weT[G1 * k * P : G1 * (k + 1) * P, :].rearrange(
                        "(f p) r -> p f r", p=P
                    ),
                )
                we_t.append(t)
            wt_t = []
            for k in range(NC_T // G2):
                t = w_pool.tile([P, G2 * F_OUT], mm_dt, tag="wt", name=f"wt{k}")
                nc.sync.dma_start(
                    out=t[:].rearrange("p (c o) -> p c o", o=F_OUT),
                    in_=wT[G2 * k * P : G2 * (k + 1) * P, :].rearrange(
                        "(c p) o -> p c o", p=P
                    ),
                )
                wt_t.append(t)

            # MM1: psum_h[r] = sum_f we[f, r-block].T @ xt[f]
            psum_h = [
                psum_pool.tile([P, B_SH], dt.float32, tag="acc", name=f"ph{r}")
                for r in range(NR)
            ]

            # PE warm-up: ~4us of garbage matmuls starting right after the
            # memset so the HAM clock gate is at 2.4 GHz when real matmuls
            # start.  Results land in psum_h[0] partitions 0-7 and are wiped
            # by MM1's start=True.
            for _ in range(26):
                nc.tensor.matmul(
                    out=psum_h[0][0:8, :],
                    lhsT=wsrc[:, 0:8],
                    rhs=wsrc[:],
                    start=True,
                    stop=True,
                )
            for f in range(NF):
                lhs_base = we_t[f // G1]
                off = (f % G1) * R_EMB
                rhs = xt_sb[:, f * B_SH : (f + 1) * B_SH]
                for r in range(NR):
                    nc.tensor.matmul(
                        out=psum_h[r][:],
                        lhsT=lhs_base[:, off + r * P : off + (r + 1) * P],
                        rhs=rhs,
                        start=(f == 0),
                        stop=(f == NF - 1),
                    )

            hT = []
            for r in range(NR):
                t = h_pool.tile([P, B_SH], mm_dt, tag="h", name=f"h{r}")
                nc.vector.tensor_scalar_max(t[:], psum_h[r][:], 0.0)
                hT.append(t)

            # MM2: psum_o[o] = sum_c wt[c, o-block].T @ x2t[c]
            psum_o = [
                psum_pool.tile([P, B_SH], dt.float32, tag="acc", name=f"po{o}")
                for o in range(NO)
            ]
            for c in range(NC_T):
                lhs_base = wt_t[c // G2]
                off = (c % G2) * F_OUT
                if c < NF:
                    rhs = xt_sb[:, c * B_SH : (c + 1) * B_SH]
                else:
                    rhs = hT[c - NF][:]
                for o in range(NO):
                    nc.tensor.matmul(
                        out=psum_o[o][:],
                        lhsT=lhs_base[:, off + o * P : off + (o + 1) * P],
                        rhs=rhs,
                        start=(c == 0),
                        stop=(c == NC_T - 1),
                    )

            for o in range(NO):
                t = out_pool.tile([P, B_SH], dt.float32, tag="ot", name=f"ot{o}")
                nc.vector.tensor_scalar_add(t[:], psum_o[o][:], bias_sb[:, o : o + 1])
                # alternate stores across the two HWDGE rings
                ring = nc.scalar if o % 2 == 0 else nc.sync
                ring.dma_start(out=outT[o * P : (o + 1) * P, :], in_=t[:])

    _split_excess_waits(nc, mybir)
    return nc


def kernel(
    x,
    embed_rows,
    embed_cols,
    embed_vals,
    w_rows,
    w_cols,
    w_vals,
    bias_idx,
    bias_vals,
):
    from concourse.bass_utils import run_bass_kernel_spmd

    mode = os.environ.get("KERNEL_MODE", "bf16")

    # --- host-side weight prep (one-time, O(nnz)) --------------------------
    weT = (
        np.bincount(
            embed_cols.astype(np.int64) * R_EMB + embed_rows.astype(np.int64),
            weights=embed_vals.astype(np.float64),
            minlength=F_IN * R_EMB,
        )
        .reshape(F_IN, R_EMB)
        .astype(np.float32)
    )
    wT = (
        np.bincount(
            w_cols.astype(np.int64) * F_OUT + w_rows.astype(np.int64),
            weights=w_vals.astype(np.float64),
            minlength=F_MID * F_OUT,
        )
        .reshape(F_MID, F_OUT)
        .astype(np.float32)
    )
    bias = np.bincount(
        bias_idx.astype(np.int64), weights=bias_vals.astype(np.float64), minlength=F_OUT
    ).astype(np.float32)
    biasT = np.ascontiguousarray(bias.reshape(NO, P).T)

    xT = np.ascontiguousarray(x.T.astype(np.float32))
    if mode == "bf16":
        import ml_dtypes

        np_dt = ml_dtypes.bfloat16
        xT = xT.astype(np_dt)
        weT = weT.astype(np_dt)
        wT = wT.astype(np_dt)

    key = ("nc", mode)
    if key not in _cache:
        _cache[key] = _build(mode)
    nc = _cache[key]

    in_maps = [
        {
            "xT": np.ascontiguousarray(xT[:, j * B_SH : (j + 1) * B_SH]),
            "weT": weT,
            "wT": wT,
            "biasT": biasT,
        }
        for j in range(N_CORES)
    ]

    trace = bool(os.environ.get("KERNEL_TRACE"))
    kw = {}
    if trace:
        import concourse.bass_utils as bu

        bu.upload_artifacts = lambda t: t  # no artifact store in this container
        kw = dict(trace=True, tmpdir=os.environ.get("KERNEL_TRACE_DIR") or None)

    res = run_bass_kernel_spmd(nc, in_maps, list(range(N_CORES)), **kw)
    if trace:
        _cache["last_result"] = res

    out = np.empty((B, F_OUT), np.float32)
    for j in range(N_CORES):
        out[j * B_SH : (j + 1) * B_SH, :] = res.results[j]["outT"].T
    return out
